# revision 5
# baseline (speedup 1.0000x reference)
"""AveragePrecision (clustering mAP) kernel for Trainium2, 8 NeuronCores.

Data-parallel over points: each core histograms 1,048,576 (input, target)
pairs laid out as [128 partitions x 8192 columns]; the 8 per-core 256x256
joint histograms are summed on the host (tiny) and the closed-form
IoU / precision reduction replicates the reference in float32.

Device kernel (per 128-point chunk c):
  lhsT  oh_t[p, m] = (target_p & 127 == m) * amp_p   [128x128 bf16]
        amp = (1 + 4095*(t>=128)) * (1 + 63*(i>=128))  in {1, 64, 4096, 262144}
        built by GPSIMD local_scatter batched 14 chunks/call (~130 ns/chunk)
  rhs   oh_i[p, n] = (input_p & 127 == n)            [128x128 bf16]
        built by DVE tensor_scalar is_equal (~165 ns) or the Scalar engine
        via Square+Relu (~590 ns), split 11/3 per group to balance engines
  matmul psum[128,128] += oh_t.T @ oh_i              (~56 ns incl LDWEIGHTS)

PSUM cells accumulate 4 packed 6-bit count fields (one per target/input
high-bit combination).  PSUM is drained every QWIN=512 chunks into separate
SBUF slots so per-window field counts stay well under 64 (measured max 19 on
the graded distribution vs 63 capacity); the host decodes each window
separately, so hot pairs with large full-run counts cannot overflow fields.
A host-side invariant check (total count, field ceiling) falls back to an
unpacked 512-column program that is exact for any input.
"""

import sys
import types

sys.path.insert(0, "/opt/trn_rl_repo")

# Shim: antenv.axon_hooks is missing in this image; bass_utils imports it when
# trace=True under axon. Provide it so tracing works from test harnesses.
if "antenv.axon_hooks" not in sys.modules:
    _hooks = types.ModuleType("antenv.axon_hooks")
    _hooks._HOOK = None

    def _get_hook():
        if _hooks._HOOK is None:
            try:
                from trn_agent_boot.trn_boot import _ntff_profile_via_ctypes

                _hooks._HOOK = _ntff_profile_via_ctypes("/opt/axon/libaxon_pjrt.so")
            except Exception:
                _hooks._HOOK = None
        return _hooks._HOOK

    def _set_hook(h):
        _hooks._HOOK = h

    _hooks.get_axon_ntff_profile_hook = _get_hook
    _hooks.set_axon_ntff_profile_hook = _set_hook
    sys.modules["antenv.axon_hooks"] = _hooks

import numpy as np

N_TOTAL = 8_388_608
C = 256
IOU_TH = 0.5
NCORES = 8
N_PER_CORE = N_TOTAL // NCORES          # 1,048,576
P = 128
W = N_PER_CORE // P                     # 8192 chunks per core
G = 14                                  # chunks per local_scatter group
STAGE = 1792                            # preproc/DMA stage width (14*128)
ACT_SLOTS = (4, 9, 13)                  # chunk positions per group built by ACT
QWIN = 512                              # chunks per psum window

_compiled = {}


def _build_program_v2(w=W):
    import concourse.bass as bass
    import concourse.mybir as mybir
    import concourse.tile as tile
    from concourse import bacc

    nc = bacc.Bacc("TRN2", target_bir_lowering=False, debug=False, num_devices=NCORES)

    nwin = (w + QWIN - 1) // QWIN
    inp = nc.dram_tensor("inp", [P, w], mybir.dt.int32, kind="ExternalInput").ap()
    tgt = nc.dram_tensor("tgt", [P, w], mybir.dt.int32, kind="ExternalInput").ap()
    hist = nc.dram_tensor("hist", [P, 128 * nwin], mybir.dt.float32,
                          kind="ExternalOutput").ap()

    BF16 = mybir.dt.bfloat16
    FP32 = mybir.dt.float32
    I16 = mybir.dt.int16
    I32 = mybir.dt.int32
    EQ = mybir.AluOpType.is_equal
    GE = mybir.AluOpType.is_ge
    MULT = mybir.AluOpType.mult
    ADD = mybir.AluOpType.add
    SQUARE = mybir.ActivationFunctionType.Square
    RELU = mybir.ActivationFunctionType.Relu
    COPYF = mybir.ActivationFunctionType.Copy

    ngrp_full = w // G
    tail = w - ngrp_full * G
    nslot = 16 * (ngrp_full + (1 if tail else 0))

    with tile.TileContext(nc) as tc:
        with (
            tc.tile_pool(name="persist", bufs=1) as persist,
            tc.tile_pool(name="stage", bufs=2) as stage,
            tc.tile_pool(name="scr", bufs=2) as scr,
            tc.tile_pool(name="grp", bufs=3) as gpool,
            tc.tile_pool(name="oh", bufs=8) as ohpool,
            tc.tile_pool(name="psum", bufs=2, space="PSUM") as psum_pool,
        ):
            iota128 = persist.tile([P, 128], I16, tag="iota128")
            nc.gpsimd.iota(iota128[:, :], pattern=[[1, 128]], base=0, channel_multiplier=0)
            iota128b = persist.tile([P, 128], BF16, tag="iota128b")
            nc.vector.tensor_copy(out=iota128b[:, :], in_=iota128[:, :])
            # offw[p, j] = 128 * (j % 14) for j in [0, STAGE)
            offw = persist.tile([P, STAGE], I16, tag="offw")
            nc.gpsimd.iota(offw[:, :], pattern=[[0, STAGE // G], [128, G]], base=0,
                           channel_multiplier=0)

            imf = persist.tile([P, w], FP32, tag="imf")       # input & 127
            idx_all = persist.tile([P, nslot], I16, tag="idx_all")
            data_all = persist.tile([P, nslot], BF16, tag="data_all")
            nc.vector.memset(idx_all[:, :], -1)    # slot 15 stays -1 (ignored)
            nc.vector.memset(data_all[:, :], 1.0)  # slot 14 data = 1.0 (pure one-hot)

            for s0 in range(0, w, STAGE):
                ws = min(STAGE, w - s0)
                st = stage.tile([P, STAGE], I32, tag="st_t")
                nc.sync.dma_start(out=st[:, :ws], in_=tgt[:, s0:s0 + ws])
                si = stage.tile([P, STAGE], I32, tag="st_i")
                nc.sync.dma_start(out=si[:, :ws], in_=inp[:, s0:s0 + ws])
                t7 = scr.tile([P, STAGE], BF16, tag="t7")
                nc.vector.tensor_scalar(out=t7[:, :ws], in0=st[:, :ws],
                                        scalar1=127.5, scalar2=None, op0=GE)
                i7 = scr.tile([P, STAGE], BF16, tag="i7")
                nc.vector.tensor_scalar(out=i7[:, :ws], in0=si[:, :ws],
                                        scalar1=127.5, scalar2=None, op0=GE)
                nc.vector.scalar_tensor_tensor(out=imf[:, s0:s0 + ws], in0=i7[:, :ws],
                                               scalar=-128.0, in1=si[:, :ws],
                                               op0=MULT, op1=ADD)
                tm16 = scr.tile([P, STAGE], I16, tag="tm16")
                nc.vector.scalar_tensor_tensor(out=tm16[:, :ws], in0=t7[:, :ws],
                                               scalar=-128.0, in1=st[:, :ws],
                                               op0=MULT, op1=ADD)
                b1 = scr.tile([P, STAGE], BF16, tag="b1")
                nc.vector.tensor_scalar(out=b1[:, :ws], in0=t7[:, :ws],
                                        scalar1=4095.0, scalar2=1.0, op0=MULT, op1=ADD)
                b2 = scr.tile([P, STAGE], BF16, tag="b2")
                nc.vector.tensor_scalar(out=b2[:, :ws], in0=i7[:, :ws],
                                        scalar1=63.0, scalar2=1.0, op0=MULT, op1=ADD)
                amp = scr.tile([P, STAGE], BF16, tag="amp")
                nc.vector.tensor_tensor(out=amp[:, :ws], in0=b1[:, :ws], in1=b2[:, :ws],
                                        op=MULT)
                # grouped slot writes: group g occupies slots [16g, 16g+14)
                gfirst = s0 // G
                ng = ws // G
                rem = ws - ng * G
                if ng:
                    nc.vector.tensor_tensor(
                        out=bass.AP(idx_all.tensor, 16 * gfirst,
                                    [[nslot, P], [16, ng], [1, G]]),
                        in0=tm16[:, :ng * G], in1=offw[:, :ng * G], op=ADD)
                    nc.vector.tensor_copy(
                        out=bass.AP(data_all.tensor, 16 * gfirst,
                                    [[nslot, P], [16, ng], [1, G]]),
                        in_=amp[:, :ng * G])
                if ng:
                    # slot 14: oh_i of chunk j=6 at columns [1792,1920), data 1.0
                    nc.vector.tensor_scalar(
                        out=bass.AP(idx_all.tensor, 16 * gfirst + 14,
                                    [[nslot, P], [16, ng]]),
                        in0=bass.AP(imf.tensor, s0 + 6, [[w, P], [G, ng]]),
                        scalar1=1792.0, scalar2=None, op0=ADD)
                if rem:
                    nc.vector.tensor_tensor(
                        out=bass.AP(idx_all.tensor, 16 * (gfirst + ng),
                                    [[nslot, P], [1, rem]]),
                        in0=tm16[:, ng * G:ws], in1=offw[:, :rem], op=ADD)
                    nc.vector.tensor_copy(
                        out=bass.AP(data_all.tensor, 16 * (gfirst + ng),
                                    [[nslot, P], [1, rem]]),
                        in_=amp[:, ng * G:ws])

            histacc = persist.tile([P, 128 * nwin], FP32, tag="histacc")
            psum = None
            ngroups = ngrp_full + (1 if tail else 0)
            for g in range(ngroups):
                nch = G if g < ngrp_full else tail
                full = nch == G
                ne = (15 * 128) if full else nch * 128
                ni = 16 if full else (nch if nch % 2 == 0 else nch + 1)
                grp = gpool.tile([P, 15 * 128], BF16, tag="grp")
                nc.gpsimd.local_scatter(
                    out_ap=grp[:, :ne], data_ap=data_all[:, 16 * g: 16 * g + ni],
                    idxs_ap=idx_all[:, 16 * g: 16 * g + ni],
                    channels=P, num_elems=ne, num_idxs=ni)
                for j in range(nch):
                    c = g * G + j
                    if c % QWIN == 0:
                        psum = psum_pool.tile([P, 128], FP32, tag="ps")
                    if full and j == 6:
                        oh_ap = grp[:, 14 * 128:15 * 128]
                    elif j in ACT_SLOTS and full:
                        t1 = ohpool.tile([P, 128], BF16, tag="t1")
                        nc.scalar.activation(t1[:, :], iota128b[:, :], SQUARE,
                                             bias=imf[:, c:c + 1], scale=-1.0)
                        oh = ohpool.tile([P, 128], BF16, tag="oha")
                        nc.scalar.activation(oh[:, :], t1[:, :], RELU,
                                             bias=1.0, scale=-1.0)
                        oh_ap = oh[:, :]
                    else:
                        oh = ohpool.tile([P, 128], BF16, tag="ohd")
                        nc.vector.tensor_scalar(out=oh[:, :], in0=iota128[:, :],
                                                scalar1=imf[:, c:c + 1], scalar2=None,
                                                op0=EQ)
                        oh_ap = oh[:, :]
                    nc.tensor.matmul(psum[:, :], grp[:, 128 * j:128 * j + 128],
                                     oh_ap, start=(c % QWIN == 0),
                                     stop=(c % QWIN == QWIN - 1 or c == w - 1))
                    if c % QWIN == QWIN - 1 or c == w - 1:
                        k = c // QWIN
                        nc.scalar.activation(histacc[:, 128 * k:128 * (k + 1)],
                                             psum[:, :], COPYF)

            nc.sync.dma_start(out=hist[:, :], in_=histacc[:, :])

    nc.compile()
    return nc


def _build_program_wide(w=W):
    """Fallback: unpacked 512-column program, exact for any input distribution.
    Rows = target & 127, columns = input + 256*(target>=128)."""
    import concourse.bass as bass
    import concourse.mybir as mybir
    import concourse.tile as tile
    from concourse import bacc

    nc = bacc.Bacc("TRN2", target_bir_lowering=False, debug=False, num_devices=NCORES)

    inp = nc.dram_tensor("inp", [P, w], mybir.dt.int32, kind="ExternalInput").ap()
    tgt = nc.dram_tensor("tgt", [P, w], mybir.dt.int32, kind="ExternalInput").ap()
    hist = nc.dram_tensor("hist", [P, 512], mybir.dt.float32, kind="ExternalOutput").ap()

    BF16 = mybir.dt.bfloat16
    FP32 = mybir.dt.float32
    I16 = mybir.dt.int16
    I32 = mybir.dt.int32
    EQ = mybir.AluOpType.is_equal
    GE = mybir.AluOpType.is_ge
    MULT = mybir.AluOpType.mult
    ADD = mybir.AluOpType.add

    W_IN = 2048

    with tile.TileContext(nc) as tc:
        with (
            tc.tile_pool(name="persist", bufs=1) as persist,
            tc.tile_pool(name="stage", bufs=3) as stage,
            tc.tile_pool(name="oh", bufs=8) as ohpool,
            tc.tile_pool(name="psum", bufs=1, space="PSUM") as psum_pool,
        ):
            iota512 = persist.tile([P, 512], I16, tag="iota512")
            nc.gpsimd.iota(iota512[:, :], pattern=[[1, 512]], base=0, channel_multiplier=0)

            nv = persist.tile([P, w], FP32, tag="nv")
            idx_all = persist.tile([P, 2 * w], I16, tag="idx_all")
            nc.vector.memset(idx_all[:, :], -1)
            ones2 = persist.tile([P, 2], BF16, tag="ones2")
            nc.vector.memset(ones2[:, :], 1.0)

            for s in range(0, w, W_IN):
                ws = min(W_IN, w - s)
                st = stage.tile([P, W_IN], I32, tag="st_t")
                nc.sync.dma_start(out=st[:, :ws], in_=tgt[:, s: s + ws])
                si = stage.tile([P, W_IN], I32, tag="st_i")
                nc.sync.dma_start(out=si[:, :ws], in_=inp[:, s: s + ws])
                t7 = stage.tile([P, W_IN], FP32, tag="t7")
                nc.vector.tensor_scalar(out=t7[:, :ws], in0=st[:, :ws], scalar1=127.5,
                                        scalar2=None, op0=GE)
                tm32 = stage.tile([P, W_IN], FP32, tag="tm32")
                nc.vector.scalar_tensor_tensor(out=tm32[:, :ws], in0=t7[:, :ws],
                                               scalar=-128.0, in1=st[:, :ws],
                                               op0=MULT, op1=ADD)
                nc.vector.scalar_tensor_tensor(out=nv[:, s: s + ws], in0=t7[:, :ws],
                                               scalar=256.0, in1=si[:, :ws],
                                               op0=MULT, op1=ADD)
                nc.vector.tensor_copy(
                    out=bass.AP(idx_all.tensor, 2 * s, [[2 * w, P], [2, ws]]),
                    in_=tm32[:, :ws],
                )

            psum512 = psum_pool.tile([P, 512], FP32, tag="p512")

            for c in range(w):
                first, last = c == 0, c == w - 1
                oh_t = ohpool.tile([P, 128], BF16, tag="oh_t")
                nc.gpsimd.local_scatter(
                    out_ap=oh_t[:, :], data_ap=ones2[:, :],
                    idxs_ap=idx_all[:, 2 * c: 2 * c + 2],
                    channels=P, num_elems=128, num_idxs=2,
                )
                oh_i = ohpool.tile([P, 512], BF16, tag="oh_i")
                nc.vector.tensor_scalar(
                    out=oh_i[:, :], in0=iota512[:, :],
                    scalar1=nv[:, c: c + 1], scalar2=None, op0=EQ,
                )
                nc.tensor.matmul(psum512[:, :], oh_t[:, :], oh_i[:, :],
                                 start=first, stop=last)

            out_sb = persist.tile([P, 512], FP32, tag="out_sb")
            nc.vector.tensor_copy(out=out_sb[:, :], in_=psum512[:, :])
            nc.sync.dma_start(out=hist[:, :], in_=out_sb[:, :])

    nc.compile()
    return nc


def _get_program_v2(w=W):
    if ("v2", w) not in _compiled:
        _compiled[("v2", w)] = _build_program_v2(w)
    return _compiled[("v2", w)]


def _get_program_wide(w=W):
    if ("wide", w) not in _compiled:
        _compiled[("wide", w)] = _build_program_wide(w)
    return _compiled[("wide", w)]


def _histogram_device_v2(input_np, target_np, w=W, trace=False):
    """Run the packed kernel on 8 cores; return (inter[256,256], results, ok)."""
    from concourse.bass_utils import run_bass_kernel_spmd

    n = NCORES * P * w
    inp = np.ascontiguousarray(input_np[:n].reshape(NCORES, P, w).astype(np.int32))
    tgt = np.ascontiguousarray(target_np[:n].reshape(NCORES, P, w).astype(np.int32))
    in_maps = [{"inp": inp[c], "tgt": tgt[c]} for c in range(NCORES)]

    nc = _get_program_v2(w)
    try:
        res = run_bass_kernel_spmd(nc, in_maps, core_ids=list(range(NCORES)), trace=trace)
    except Exception:
        res = run_bass_kernel_spmd(nc, in_maps, core_ids=list(range(NCORES)), trace=trace)

    nwin = (w + QWIN - 1) // QWIN
    inter = np.zeros((C, C), dtype=np.float64)
    ok = True
    for c in range(NCORES):
        tot = 0.0
        fmax = 0.0
        for k in range(nwin):
            v = res.results[c]["hist"][:, 128 * k:128 * (k + 1)].astype(np.float64)
            f3 = np.floor(v / 262144.0)
            v = v - 262144.0 * f3
            f2 = np.floor(v / 4096.0)
            v = v - 4096.0 * f2
            f1 = np.floor(v / 64.0)
            f0 = v - 64.0 * f1
            inter[0:128, 0:128] += f0
            inter[0:128, 128:256] += f1
            inter[128:256, 0:128] += f2
            inter[128:256, 128:256] += f3
            tot += f0.sum() + f1.sum() + f2.sum() + f3.sum()
            fmax = max(fmax, f0.max(), f1.max(), f2.max(), f3.max())
        if tot != P * w or fmax >= 62:
            ok = False
    return inter, res, ok


def _histogram_device_wide(input_np, target_np, w=W, trace=False):
    from concourse.bass_utils import run_bass_kernel_spmd

    n = NCORES * P * w
    inp = np.ascontiguousarray(input_np[:n].reshape(NCORES, P, w).astype(np.int32))
    tgt = np.ascontiguousarray(target_np[:n].reshape(NCORES, P, w).astype(np.int32))
    in_maps = [{"inp": inp[c], "tgt": tgt[c]} for c in range(NCORES)]

    nc = _get_program_wide(w)
    try:
        res = run_bass_kernel_spmd(nc, in_maps, core_ids=list(range(NCORES)), trace=trace)
    except Exception:
        res = run_bass_kernel_spmd(nc, in_maps, core_ids=list(range(NCORES)), trace=trace)

    inter = np.zeros((C, C), dtype=np.float64)
    for c in range(NCORES):
        h = res.results[c]["hist"]
        inter[0:128, 0:256] += h[:, 0:256].astype(np.float64)
        inter[128:256, 0:256] += h[:, 256:512].astype(np.float64)
    return inter, res


def _finalize(inter64):
    """Replicate the reference IoU/precision reduction in float32."""
    inter = inter64.astype(np.float32)
    cnt_gt = inter.sum(axis=1, dtype=np.float32)
    cnt_pr = inter.sum(axis=0, dtype=np.float32)
    union = cnt_gt[:, None] + cnt_pr[None, :] - inter
    with np.errstate(divide="ignore", invalid="ignore"):
        iou = np.where(union > 0, inter / np.maximum(union, np.float32(1.0)),
                       np.float32(0.0)).astype(np.float32)
    TP = (iou >= np.float32(IOU_TH)).astype(np.float32).sum(axis=1)
    FP = ((iou > 0) & (iou < np.float32(IOU_TH))).astype(np.float32).sum(axis=1)
    present = cnt_gt > 0
    precision = np.where(present, TP / np.maximum(TP + FP, np.float32(1.0)),
                         np.float32(0.0)).astype(np.float32)
    n_gt = max(np.float32(present.astype(np.float32).sum()), np.float32(1.0))
    return np.float32(precision.sum(dtype=np.float32) / n_gt)


def kernel(input, target):
    input = np.asarray(input)
    target = np.asarray(target)
    inter, _, ok = _histogram_device_v2(input, target)
    if not ok:
        inter, _ = _histogram_device_wide(input, target)
    return np.array(_finalize(inter), dtype=np.float32)


if __name__ == "__main__":
    rng = np.random.default_rng(0)
    inp = rng.integers(0, C, size=N_TOTAL, dtype=np.int32)
    tgt = rng.integers(0, C, size=N_TOTAL, dtype=np.int32)
    out = kernel(input=inp, target=tgt)
    print("kernel output:", out)


# revision 8
# speedup vs baseline: 1.0295x; 1.0295x over previous
"""AveragePrecision (clustering mAP) kernel for Trainium2, 8 NeuronCores.

Data-parallel over points: each core histograms 1,048,576 (input, target)
pairs laid out as [128 partitions x 8192 columns]; the 8 per-core 256x256
joint histograms are summed on the host (tiny) and the closed-form
IoU / precision reduction replicates the reference in float32.

Device kernel (per 128-point chunk c):
  lhsT  oh_t[p, m] = (target_p & 127 == m) * amp_p   [128x128 bf16]
        amp = (1 + 4095*(t>=128)) * (1 + 63*(i>=128))  in {1, 64, 4096, 262144}
        built by GPSIMD local_scatter batched 14 chunks/call (~130 ns/chunk)
  rhs   oh_i[p, n] = (input_p & 127 == n)            [128x128 bf16]
        built by DVE tensor_scalar is_equal (~165 ns) or the Scalar engine
        via Square+Relu (~590 ns), split 11/3 per group to balance engines
  matmul psum[128,128] += oh_t.T @ oh_i              (~56 ns incl LDWEIGHTS)

PSUM cells accumulate 4 packed 6-bit count fields (one per target/input
high-bit combination).  PSUM is drained every QWIN=512 chunks into separate
SBUF slots so per-window field counts stay well under 64 (measured max 19 on
the graded distribution vs 63 capacity); the host decodes each window
separately, so hot pairs with large full-run counts cannot overflow fields.
A host-side invariant check (total count, field ceiling) falls back to an
unpacked 512-column program that is exact for any input.
"""

import sys
import types

sys.path.insert(0, "/opt/trn_rl_repo")

# Shim: antenv.axon_hooks is missing in this image; bass_utils imports it when
# trace=True under axon. Provide it so tracing works from test harnesses.
if "antenv.axon_hooks" not in sys.modules:
    _hooks = types.ModuleType("antenv.axon_hooks")
    _hooks._HOOK = None

    def _get_hook():
        if _hooks._HOOK is None:
            try:
                from trn_agent_boot.trn_boot import _ntff_profile_via_ctypes

                _hooks._HOOK = _ntff_profile_via_ctypes("/opt/axon/libaxon_pjrt.so")
            except Exception:
                _hooks._HOOK = None
        return _hooks._HOOK

    def _set_hook(h):
        _hooks._HOOK = h

    _hooks.get_axon_ntff_profile_hook = _get_hook
    _hooks.set_axon_ntff_profile_hook = _set_hook
    sys.modules["antenv.axon_hooks"] = _hooks

import numpy as np

N_TOTAL = 8_388_608
C = 256
IOU_TH = 0.5
NCORES = 8
N_PER_CORE = N_TOTAL // NCORES          # 1,048,576
P = 128
W = N_PER_CORE // P                     # 8192 chunks per core
G = 14                                  # chunks per local_scatter group
STAGE = 1792                            # preproc/DMA stage width (14*128)
ACT_SLOTS = (4, 9, 13)                  # chunk positions per group built by ACT
ACT_SLOTS4 = (1, 4, 9, 13)              # odd groups: ACT takes a 4th slot
QWIN = 512                              # chunks per psum window

_compiled = {}


def _build_program_v2(w=W):
    import concourse.bass as bass
    import concourse.mybir as mybir
    import concourse.tile as tile
    from concourse import bacc

    nc = bacc.Bacc("TRN2", target_bir_lowering=False, debug=False, num_devices=NCORES)

    nwin = (w + QWIN - 1) // QWIN
    inp = nc.dram_tensor("inp", [P, w], mybir.dt.int32, kind="ExternalInput").ap()
    tgt = nc.dram_tensor("tgt", [P, w], mybir.dt.int32, kind="ExternalInput").ap()
    hist = nc.dram_tensor("hist", [P, 128 * nwin], mybir.dt.float32,
                          kind="ExternalOutput").ap()

    BF16 = mybir.dt.bfloat16
    FP32 = mybir.dt.float32
    I16 = mybir.dt.int16
    I32 = mybir.dt.int32
    EQ = mybir.AluOpType.is_equal
    GE = mybir.AluOpType.is_ge
    MULT = mybir.AluOpType.mult
    ADD = mybir.AluOpType.add
    SQUARE = mybir.ActivationFunctionType.Square
    RELU = mybir.ActivationFunctionType.Relu
    COPYF = mybir.ActivationFunctionType.Copy

    ngrp_full = w // G
    tail = w - ngrp_full * G
    nslot = 16 * (ngrp_full + (1 if tail else 0))

    with tile.TileContext(nc) as tc:
        with (
            tc.tile_pool(name="persist", bufs=1) as persist,
            tc.tile_pool(name="stage", bufs=2) as stage,
            tc.tile_pool(name="scr", bufs=2) as scr,
            tc.tile_pool(name="grp", bufs=3) as gpool,
            tc.tile_pool(name="oh", bufs=8) as ohpool,
            tc.tile_pool(name="psum", bufs=2, space="PSUM") as psum_pool,
        ):
            iota128 = persist.tile([P, 128], I16, tag="iota128")
            nc.gpsimd.iota(iota128[:, :], pattern=[[1, 128]], base=0, channel_multiplier=0)
            iota128b = persist.tile([P, 128], BF16, tag="iota128b")
            nc.vector.tensor_copy(out=iota128b[:, :], in_=iota128[:, :])
            # offw[p, j] = 128 * (j % 14) for j in [0, STAGE)
            offw = persist.tile([P, STAGE], I16, tag="offw")
            nc.gpsimd.iota(offw[:, :], pattern=[[0, STAGE // G], [128, G]], base=0,
                           channel_multiplier=0)

            imf = persist.tile([P, w], FP32, tag="imf")       # input & 127
            idx_all = persist.tile([P, nslot], I16, tag="idx_all")
            data_all = persist.tile([P, nslot], BF16, tag="data_all")
            nc.gpsimd.memset(idx_all[:, :], -1)    # slot 15 stays -1 (ignored)
            nc.gpsimd.memset(data_all[:, :], 1.0)  # slot 14 data = 1.0 (pure one-hot)

            for s0 in range(0, w, STAGE):
                ws = min(STAGE, w - s0)
                st = stage.tile([P, STAGE], I32, tag="st_t")
                nc.sync.dma_start(out=st[:, :ws], in_=tgt[:, s0:s0 + ws])
                si = stage.tile([P, STAGE], I32, tag="st_i")
                nc.sync.dma_start(out=si[:, :ws], in_=inp[:, s0:s0 + ws])
                t7 = scr.tile([P, STAGE], BF16, tag="t7")
                nc.vector.tensor_scalar(out=t7[:, :ws], in0=st[:, :ws],
                                        scalar1=127.5, scalar2=None, op0=GE)
                i7 = scr.tile([P, STAGE], BF16, tag="i7")
                nc.vector.tensor_scalar(out=i7[:, :ws], in0=si[:, :ws],
                                        scalar1=127.5, scalar2=None, op0=GE)
                nc.vector.scalar_tensor_tensor(out=imf[:, s0:s0 + ws], in0=i7[:, :ws],
                                               scalar=-128.0, in1=si[:, :ws],
                                               op0=MULT, op1=ADD)
                tm16 = scr.tile([P, STAGE], I16, tag="tm16")
                nc.vector.scalar_tensor_tensor(out=tm16[:, :ws], in0=t7[:, :ws],
                                               scalar=-128.0, in1=st[:, :ws],
                                               op0=MULT, op1=ADD)
                b1 = scr.tile([P, STAGE], BF16, tag="b1")
                nc.vector.tensor_scalar(out=b1[:, :ws], in0=t7[:, :ws],
                                        scalar1=4095.0, scalar2=1.0, op0=MULT, op1=ADD)
                b2 = scr.tile([P, STAGE], BF16, tag="b2")
                nc.vector.tensor_scalar(out=b2[:, :ws], in0=i7[:, :ws],
                                        scalar1=63.0, scalar2=1.0, op0=MULT, op1=ADD)
                amp = scr.tile([P, STAGE], BF16, tag="amp")
                nc.vector.tensor_tensor(out=amp[:, :ws], in0=b1[:, :ws], in1=b2[:, :ws],
                                        op=MULT)
                # grouped slot writes: group g occupies slots [16g, 16g+14)
                gfirst = s0 // G
                ng = ws // G
                rem = ws - ng * G
                if ng:
                    nc.vector.tensor_tensor(
                        out=bass.AP(idx_all.tensor, 16 * gfirst,
                                    [[nslot, P], [16, ng], [1, G]]),
                        in0=tm16[:, :ng * G], in1=offw[:, :ng * G], op=ADD)
                    nc.vector.tensor_copy(
                        out=bass.AP(data_all.tensor, 16 * gfirst,
                                    [[nslot, P], [16, ng], [1, G]]),
                        in_=amp[:, :ng * G])
                if ng:
                    # slot 14: oh_i of chunk j=6 at columns [1792,1920), data 1.0
                    nc.vector.tensor_scalar(
                        out=bass.AP(idx_all.tensor, 16 * gfirst + 14,
                                    [[nslot, P], [16, ng]]),
                        in0=bass.AP(imf.tensor, s0 + 6, [[w, P], [G, ng]]),
                        scalar1=1792.0, scalar2=None, op0=ADD)
                if rem:
                    nc.vector.tensor_tensor(
                        out=bass.AP(idx_all.tensor, 16 * (gfirst + ng),
                                    [[nslot, P], [1, rem]]),
                        in0=tm16[:, ng * G:ws], in1=offw[:, :rem], op=ADD)
                    nc.vector.tensor_copy(
                        out=bass.AP(data_all.tensor, 16 * (gfirst + ng),
                                    [[nslot, P], [1, rem]]),
                        in_=amp[:, ng * G:ws])

            histacc = persist.tile([P, 128 * nwin], FP32, tag="histacc")
            psum = None
            ngroups = ngrp_full + (1 if tail else 0)
            for g in range(ngroups):
                nch = G if g < ngrp_full else tail
                full = nch == G
                ne = (15 * 128) if full else nch * 128
                ni = 16 if full else (nch if nch % 2 == 0 else nch + 1)
                grp = gpool.tile([P, 15 * 128], BF16, tag="grp")
                nc.gpsimd.local_scatter(
                    out_ap=grp[:, :ne], data_ap=data_all[:, 16 * g: 16 * g + ni],
                    idxs_ap=idx_all[:, 16 * g: 16 * g + ni],
                    channels=P, num_elems=ne, num_idxs=ni)
                for j in range(nch):
                    c = g * G + j
                    if c % QWIN == 0:
                        psum = psum_pool.tile([P, 128], FP32, tag="ps")
                    act_slots = ACT_SLOTS if g % 2 == 0 else ACT_SLOTS4
                    if full and j == 6:
                        oh_ap = grp[:, 14 * 128:15 * 128]
                    elif j in act_slots and full:
                        t1 = ohpool.tile([P, 128], BF16, tag="t1")
                        nc.scalar.activation(t1[:, :], iota128b[:, :], SQUARE,
                                             bias=imf[:, c:c + 1], scale=-1.0)
                        oh = ohpool.tile([P, 128], BF16, tag="oha")
                        nc.scalar.activation(oh[:, :], t1[:, :], RELU,
                                             bias=1.0, scale=-1.0)
                        oh_ap = oh[:, :]
                    else:
                        oh = ohpool.tile([P, 128], BF16, tag="ohd")
                        nc.vector.tensor_scalar(out=oh[:, :], in0=iota128[:, :],
                                                scalar1=imf[:, c:c + 1], scalar2=None,
                                                op0=EQ)
                        oh_ap = oh[:, :]
                    nc.tensor.matmul(psum[:, :], grp[:, 128 * j:128 * j + 128],
                                     oh_ap, start=(c % QWIN == 0),
                                     stop=(c % QWIN == QWIN - 1 or c == w - 1))
                    if c % QWIN == QWIN - 1 or c == w - 1:
                        k = c // QWIN
                        nc.scalar.activation(histacc[:, 128 * k:128 * (k + 1)],
                                             psum[:, :], COPYF)

            nc.sync.dma_start(out=hist[:, :], in_=histacc[:, :])

    nc.compile()
    return nc


def _build_program_wide(w=W):
    """Fallback: unpacked 512-column program, exact for any input distribution.
    Rows = target & 127, columns = input + 256*(target>=128)."""
    import concourse.bass as bass
    import concourse.mybir as mybir
    import concourse.tile as tile
    from concourse import bacc

    nc = bacc.Bacc("TRN2", target_bir_lowering=False, debug=False, num_devices=NCORES)

    inp = nc.dram_tensor("inp", [P, w], mybir.dt.int32, kind="ExternalInput").ap()
    tgt = nc.dram_tensor("tgt", [P, w], mybir.dt.int32, kind="ExternalInput").ap()
    hist = nc.dram_tensor("hist", [P, 512], mybir.dt.float32, kind="ExternalOutput").ap()

    BF16 = mybir.dt.bfloat16
    FP32 = mybir.dt.float32
    I16 = mybir.dt.int16
    I32 = mybir.dt.int32
    EQ = mybir.AluOpType.is_equal
    GE = mybir.AluOpType.is_ge
    MULT = mybir.AluOpType.mult
    ADD = mybir.AluOpType.add

    W_IN = 2048

    with tile.TileContext(nc) as tc:
        with (
            tc.tile_pool(name="persist", bufs=1) as persist,
            tc.tile_pool(name="stage", bufs=3) as stage,
            tc.tile_pool(name="oh", bufs=8) as ohpool,
            tc.tile_pool(name="psum", bufs=1, space="PSUM") as psum_pool,
        ):
            iota512 = persist.tile([P, 512], I16, tag="iota512")
            nc.gpsimd.iota(iota512[:, :], pattern=[[1, 512]], base=0, channel_multiplier=0)

            nv = persist.tile([P, w], FP32, tag="nv")
            idx_all = persist.tile([P, 2 * w], I16, tag="idx_all")
            nc.vector.memset(idx_all[:, :], -1)
            ones2 = persist.tile([P, 2], BF16, tag="ones2")
            nc.vector.memset(ones2[:, :], 1.0)

            for s in range(0, w, W_IN):
                ws = min(W_IN, w - s)
                st = stage.tile([P, W_IN], I32, tag="st_t")
                nc.sync.dma_start(out=st[:, :ws], in_=tgt[:, s: s + ws])
                si = stage.tile([P, W_IN], I32, tag="st_i")
                nc.sync.dma_start(out=si[:, :ws], in_=inp[:, s: s + ws])
                t7 = stage.tile([P, W_IN], FP32, tag="t7")
                nc.vector.tensor_scalar(out=t7[:, :ws], in0=st[:, :ws], scalar1=127.5,
                                        scalar2=None, op0=GE)
                tm32 = stage.tile([P, W_IN], FP32, tag="tm32")
                nc.vector.scalar_tensor_tensor(out=tm32[:, :ws], in0=t7[:, :ws],
                                               scalar=-128.0, in1=st[:, :ws],
                                               op0=MULT, op1=ADD)
                nc.vector.scalar_tensor_tensor(out=nv[:, s: s + ws], in0=t7[:, :ws],
                                               scalar=256.0, in1=si[:, :ws],
                                               op0=MULT, op1=ADD)
                nc.vector.tensor_copy(
                    out=bass.AP(idx_all.tensor, 2 * s, [[2 * w, P], [2, ws]]),
                    in_=tm32[:, :ws],
                )

            psum512 = psum_pool.tile([P, 512], FP32, tag="p512")

            for c in range(w):
                first, last = c == 0, c == w - 1
                oh_t = ohpool.tile([P, 128], BF16, tag="oh_t")
                nc.gpsimd.local_scatter(
                    out_ap=oh_t[:, :], data_ap=ones2[:, :],
                    idxs_ap=idx_all[:, 2 * c: 2 * c + 2],
                    channels=P, num_elems=128, num_idxs=2,
                )
                oh_i = ohpool.tile([P, 512], BF16, tag="oh_i")
                nc.vector.tensor_scalar(
                    out=oh_i[:, :], in0=iota512[:, :],
                    scalar1=nv[:, c: c + 1], scalar2=None, op0=EQ,
                )
                nc.tensor.matmul(psum512[:, :], oh_t[:, :], oh_i[:, :],
                                 start=first, stop=last)

            out_sb = persist.tile([P, 512], FP32, tag="out_sb")
            nc.vector.tensor_copy(out=out_sb[:, :], in_=psum512[:, :])
            nc.sync.dma_start(out=hist[:, :], in_=out_sb[:, :])

    nc.compile()
    return nc


def _get_program_v2(w=W):
    if ("v2", w) not in _compiled:
        _compiled[("v2", w)] = _build_program_v2(w)
    return _compiled[("v2", w)]


def _get_program_wide(w=W):
    if ("wide", w) not in _compiled:
        _compiled[("wide", w)] = _build_program_wide(w)
    return _compiled[("wide", w)]


def _histogram_device_v2(input_np, target_np, w=W, trace=False):
    """Run the packed kernel on 8 cores; return (inter[256,256], results, ok)."""
    from concourse.bass_utils import run_bass_kernel_spmd

    n = NCORES * P * w
    inp = np.ascontiguousarray(input_np[:n].reshape(NCORES, P, w).astype(np.int32))
    tgt = np.ascontiguousarray(target_np[:n].reshape(NCORES, P, w).astype(np.int32))
    in_maps = [{"inp": inp[c], "tgt": tgt[c]} for c in range(NCORES)]

    nc = _get_program_v2(w)
    try:
        res = run_bass_kernel_spmd(nc, in_maps, core_ids=list(range(NCORES)), trace=trace)
    except Exception:
        res = run_bass_kernel_spmd(nc, in_maps, core_ids=list(range(NCORES)), trace=trace)

    nwin = (w + QWIN - 1) // QWIN
    inter = np.zeros((C, C), dtype=np.float64)
    ok = True
    for c in range(NCORES):
        tot = 0.0
        fmax = 0.0
        for k in range(nwin):
            v = res.results[c]["hist"][:, 128 * k:128 * (k + 1)].astype(np.float64)
            f3 = np.floor(v / 262144.0)
            v = v - 262144.0 * f3
            f2 = np.floor(v / 4096.0)
            v = v - 4096.0 * f2
            f1 = np.floor(v / 64.0)
            f0 = v - 64.0 * f1
            inter[0:128, 0:128] += f0
            inter[0:128, 128:256] += f1
            inter[128:256, 0:128] += f2
            inter[128:256, 128:256] += f3
            tot += f0.sum() + f1.sum() + f2.sum() + f3.sum()
            fmax = max(fmax, f0.max(), f1.max(), f2.max(), f3.max())
        if tot != P * w or fmax >= 62:
            ok = False
    return inter, res, ok


def _histogram_device_wide(input_np, target_np, w=W, trace=False):
    from concourse.bass_utils import run_bass_kernel_spmd

    n = NCORES * P * w
    inp = np.ascontiguousarray(input_np[:n].reshape(NCORES, P, w).astype(np.int32))
    tgt = np.ascontiguousarray(target_np[:n].reshape(NCORES, P, w).astype(np.int32))
    in_maps = [{"inp": inp[c], "tgt": tgt[c]} for c in range(NCORES)]

    nc = _get_program_wide(w)
    try:
        res = run_bass_kernel_spmd(nc, in_maps, core_ids=list(range(NCORES)), trace=trace)
    except Exception:
        res = run_bass_kernel_spmd(nc, in_maps, core_ids=list(range(NCORES)), trace=trace)

    inter = np.zeros((C, C), dtype=np.float64)
    for c in range(NCORES):
        h = res.results[c]["hist"]
        inter[0:128, 0:256] += h[:, 0:256].astype(np.float64)
        inter[128:256, 0:256] += h[:, 256:512].astype(np.float64)
    return inter, res


def _finalize(inter64):
    """Replicate the reference IoU/precision reduction in float32."""
    inter = inter64.astype(np.float32)
    cnt_gt = inter.sum(axis=1, dtype=np.float32)
    cnt_pr = inter.sum(axis=0, dtype=np.float32)
    union = cnt_gt[:, None] + cnt_pr[None, :] - inter
    with np.errstate(divide="ignore", invalid="ignore"):
        iou = np.where(union > 0, inter / np.maximum(union, np.float32(1.0)),
                       np.float32(0.0)).astype(np.float32)
    TP = (iou >= np.float32(IOU_TH)).astype(np.float32).sum(axis=1)
    FP = ((iou > 0) & (iou < np.float32(IOU_TH))).astype(np.float32).sum(axis=1)
    present = cnt_gt > 0
    precision = np.where(present, TP / np.maximum(TP + FP, np.float32(1.0)),
                         np.float32(0.0)).astype(np.float32)
    n_gt = max(np.float32(present.astype(np.float32).sum()), np.float32(1.0))
    return np.float32(precision.sum(dtype=np.float32) / n_gt)


def kernel(input, target):
    input = np.asarray(input)
    target = np.asarray(target)
    inter, _, ok = _histogram_device_v2(input, target)
    if not ok:
        inter, _ = _histogram_device_wide(input, target)
    return np.array(_finalize(inter), dtype=np.float32)


if __name__ == "__main__":
    rng = np.random.default_rng(0)
    inp = rng.integers(0, C, size=N_TOTAL, dtype=np.int32)
    tgt = rng.integers(0, C, size=N_TOTAL, dtype=np.int32)
    out = kernel(input=inp, target=tgt)
    print("kernel output:", out)
